# revision 25
# baseline (speedup 1.0000x reference)
"""Trainium2 kernel for ChannelQuadLayer.

Per-pixel quadratic channel expansion + 1x1 conv:
    quad = x[:, ii] * x[:, jj]  (all 2080 upper-tri channel pairs)
    y    = concat([x, quad])    -> [B, 2144, H, W]
    out  = einsum('bchw,oc->bohw', y, fc_w)

Strategy (8 NeuronCores, batch-parallel, one sample per core):
  * The 2080 unordered channel pairs are exactly the cyclic diagonals
    d=0..32 of the 64-channel index ring: pairs {i, (i+d)%64}.
  * Host prepares 9 "rotation buffers" B_k = [roll(x,-t_k); roll(x,-u_k)]
    (128 partitions x 4096 pixels, bf16). One elementwise multiply of two
    such buffers yields TWO complete cyclic diagonals; a difference cover
    produces all diagonals 1..32 in 16 multiplies. Diagonal 0 (squares)
    and the linear rows come from buffer 0 (copy + self-multiply).
  * y-rows: 64 linear + 64 squares + 16*128 pair rows = 2176 = 17*128,
    an exact 17-chunk contraction. fc_w is permuted/padded to this row
    order on the host (duplicate pair rows get zero weight), cast bf16.
  * GEMM: out[256, 4096] = Wt[2176, 256]^T @ y[2176, 4096] on TensorE in
    bf16 (full PE rate, half the SBUF/HBM traffic of fp32), accumulating
    17 chunks into fp32 PSUM, k-outer so each y chunk is consumed right
    after its producer. The PE stream is the roofline (~58us at 2.4GHz).
  * Engine split keeps the PE gap-free: VectorE produces ALL y chunks
    (incl. chunk 0 via copy+self-mul), ScalarE only drains PSUM.
  * Startup is DMA-latency-bound, so supply is need-ordered over three
    DMA lanes: SP carries the weights in 6 chunk-ordered slices then
    late-pass buffer blocks, GPSIMD carries pass-0 buffers one by one in
    compute order, ScalarE prefetches pass 1 (its triggers are emitted
    before pass-0 drains). Pass widths [512,1024,1024,1024,512]: small
    first pass starts the PE early, small last pass shortens the tail.
"""

import sys

sys.path.insert(0, "/opt/trn_rl_repo")

import numpy as np
import ml_dtypes

import concourse.bass as bass
import concourse.tile as tile
from concourse import bacc, mybir
from concourse.bass_utils import run_bass_kernel_spmd

B, C, H, W = 8, 64, 64, 64
PIX = H * W  # 4096
OUT = 256
NCORES = 8

# rotation difference cover: ops (i,j) give diagonals D(t_j-t_i) (top half)
# and D(u_j-u_i) (bottom half); together exactly {1..32}.
T_ROT = [0, 8, 22, 24, 42, 48, 49, 57, 60]
U_ROT = [0, 59, 16, 38, 55, 22, 30, 54, 35]
# group A = buffers 0-4, group B = buffers 5-8; A-only ops first so pass 0
# can start computing while group B is still in flight.
OPS_A = [(1, 3), (2, 3), (1, 4), (2, 4), (3, 4)]
OPS_B = [(4, 5), (1, 6), (2, 6), (6, 7), (0, 7), (4, 7),
         (5, 7), (2, 8), (3, 8), (5, 8), (6, 8)]
OPS = OPS_A + OPS_B
NB = len(T_ROT)        # 9 rotation buffers
NA, NBB = 5, 4         # buffers per group
KCH = 1 + len(OPS)     # 17 contraction chunks of 128 rows
PASS_FD = [512, 1024, 1024, 1024, 512]
assert sum(PASS_FD) == PIX
NPASS = len(PASS_FD)
NW = 512               # matmul free width (one PSUM bank)
# weight DMA split (chunk boundaries): chunk 0 alone so the PE can start
# after 65KB, chunks 1-5 / 6-16 as separate contiguous tensors
W_A0, W_A1 = 1, 1 + len(OPS_A)
# pass-0 group-A buffer order in DRAM = compute order, so the two
# segments [b0,b1,b3] and [b2,b4] are contiguous single transfers
B0A_ORDER = [0, 1, 3, 2, 4]
B0A_POS = {b: i for i, b in enumerate(B0A_ORDER)}

F32 = mybir.dt.float32
BF16 = mybir.dt.bfloat16
NPBF16 = ml_dtypes.bfloat16


def row_pairs():
    """Channel pair (c1, c2) for every global y row, or ('lin', c)."""
    rows = []
    for p in range(128):  # chunk 0
        rows.append(("lin", p) if p < 64 else (p - 64, p - 64))
    for (i, j) in OPS:
        for p in range(128):
            if p < 64:
                c1, c2 = (p + T_ROT[i]) % 64, (p + T_ROT[j]) % 64
            else:
                c1, c2 = (p - 64 + U_ROT[i]) % 64, (p - 64 + U_ROT[j]) % 64
            rows.append((min(c1, c2), max(c1, c2)))
    return rows


def build_wt(fc_w):
    """Permute fc_w [OUT, 2144] into Wt [KCH, 128, OUT] matching y rows."""
    ii, jj = np.triu_indices(C)
    pair2col = {(a, b): C + k for k, (a, b) in enumerate(zip(ii, jj))}
    wt = np.zeros((KCH * 128, OUT), np.float32)
    seen = set()
    for g, r in enumerate(row_pairs()):
        if r[0] == "lin":
            wt[g] = fc_w[:, r[1]]
        elif r not in seen:
            seen.add(r)
            wt[g] = fc_w[:, pair2col[r]]
    assert len(seen) == C * (C + 1) // 2
    return np.ascontiguousarray(wt.reshape(KCH, 128, OUT))


_cached = None


def _build_module():
    global _cached
    if _cached is not None:
        return _cached
    nc = bacc.Bacc("TRN2", target_bir_lowering=False, debug=False,
                   num_devices=NCORES)
    # grouped rotation buffers, pass-major so each pass is one contiguous DMA
    ba_d = nc.dram_tensor("ba", [128, NA * PIX], BF16, kind="ExternalInput")
    bb_d = nc.dram_tensor("bb", [128, NBB * PIX], BF16, kind="ExternalInput")
    # weights as three partition-major tensors so each transfer has wide
    # contiguous rows AND arrives in chunk-need order
    wt0_d = nc.dram_tensor("wt0", [128, W_A0 * OUT], BF16,
                           kind="ExternalInput")
    wta_d = nc.dram_tensor("wta", [128, (W_A1 - W_A0) * OUT], BF16,
                           kind="ExternalInput")
    wtb_d = nc.dram_tensor("wtb", [128, (KCH - W_A1) * OUT], BF16,
                           kind="ExternalInput")
    out_d = nc.dram_tensor("out", [2, 128, PIX], BF16, kind="ExternalOutput")

    with tile.TileContext(nc) as tc:
        with tc.tile_pool(name="wt", bufs=1) as wt_pool, \
             tc.tile_pool(name="warm", bufs=1) as warm_pool, \
             tc.tile_pool(name="ba", bufs=3) as ba_pool, \
             tc.tile_pool(name="bb", bufs=3) as bb_pool, \
             tc.tile_pool(name="y", bufs=8) as y_pool, \
             tc.tile_pool(name="ostage", bufs=4) as o_pool, \
             tc.tile_pool(name="psum", bufs=8, space="PSUM") as ps_pool:

            wt_t = wt_pool.tile([128, KCH * OUT], BF16, name="wtt")

            ba_t = [None] * NPASS
            bb_t = [None] * NPASS

            def emit_dma(ps, eng_a, eng_b):
                """Input DMAs for pass ps (group A / group B lanes)."""
                fd = PASS_FD[ps]
                offs = sum(PASS_FD[:ps])
                ba_t[ps] = ba_pool.tile([128, NA * 1024], BF16, tag="ba",
                                        name=f"ba{ps}")
                bb_t[ps] = bb_pool.tile([128, NBB * 1024], BF16, tag="bb",
                                        name=f"bb{ps}")
                a0, b0 = NA * offs, NBB * offs
                eng_a.dma_start(ba_t[ps][:, :NA * fd],
                                ba_d.ap()[:, a0:a0 + NA * fd])
                eng_b.dma_start(bb_t[ps][:, :NBB * fd],
                                bb_d.ap()[:, b0:b0 + NBB * fd])

            # DMA bandwidth is shared per-packet across queues and prefers
            # wide rows, so: ONE strictly-ordered input stream on the sync
            # lane (queue order = need order = perfect priority), with only
            # the two bulk weight blocks on the scalar lane, whose 2.5-5.6KB
            # rows hold their own in arbitration while pass-0 buffers flow.
            fd0 = PASS_FD[0]
            ba_t[0] = ba_pool.tile([128, NA * 1024], BF16, tag="ba", name="ba0")
            bb_t[0] = bb_pool.tile([128, NBB * 1024], BF16, tag="bb", name="bb0")

            nc.sync.dma_start(wt_t[:, :W_A0 * OUT], wt0_d.ap())
            nc.sync.dma_start(ba_t[0][:, :fd0], ba_d.ap()[:, :fd0])
            nc.sync.dma_start(ba_t[0][:, fd0:3 * fd0],
                              ba_d.ap()[:, fd0:3 * fd0])
            nc.sync.dma_start(wt_t[:, W_A0 * OUT:W_A1 * OUT], wta_d.ap())
            nc.sync.dma_start(ba_t[0][:, 3 * fd0:5 * fd0],
                              ba_d.ap()[:, 3 * fd0:5 * fd0])
            nc.sync.dma_start(bb_t[0][:, :NBB * fd0],
                              bb_d.ap()[:, :NBB * fd0])
            nc.scalar.dma_start(wt_t[:, W_A1 * OUT:], wtb_d.ap())

            # warm-up: dummy matmuls on a zeroed tile keep the PE busy
            # through the DMA-latency window before the first real chunk,
            # so the p-state ramp completes before real work arrives and
            # the first matmuls issue at full rate. Sized to end just as
            # the first real y chunk becomes ready (~11.5us).
            warm_t = warm_pool.tile([128, 512], BF16, name="warm")
            nc.gpsimd.memset(warm_t[:, :], 0)
            warm_ps = ps_pool.tile([128, NW], F32, tag="ps", name="warmps")
            for w in range(5):
                nc.tensor.matmul(warm_ps[:, :NW], warm_t[:, :128],
                                 warm_t[:, :NW], start=True, stop=True)
            for w in range(6):
                nc.tensor.matmul(warm_ps[:, :128], warm_t[:, :128],
                                 warm_t[:, :128], start=True, stop=True)

            for ps in range(NPASS):
                fd = PASS_FD[ps]
                off = sum(PASS_FD[:ps])
                nt = fd // NW
                bat, bbt = ba_t[ps], bb_t[ps]

                psum = [ps_pool.tile([128, NW], F32, tag="ps",
                                     name=f"ps{ps}_{g}")
                        for g in range(2 * nt)]

                for k in range(KCH):
                    yk = y_pool.tile([128, 1024], BF16, tag="y",
                                     name=f"y{ps}_{k}")
                    if k == 0:
                        # linear rows + squares, both from the resident b0
                        nc.vector.tensor_copy(yk[0:64, :fd], bat[0:64, :fd])
                        nc.vector.tensor_mul(yk[64:128, :fd],
                                             bat[64:128, :fd],
                                             bat[64:128, :fd])
                    else:
                        i, j = OPS[k - 1]

                        def src(i):
                            if i < NA:
                                p = B0A_POS[i] if ps == 0 else i
                                return bat[:, p * fd:(p + 1) * fd]
                            return bbt[:, (i - NA) * fd:(i - NA + 1) * fd]

                        nc.vector.tensor_mul(yk[:, :fd], src(i), src(j))
                    # final chunk of the final pass: m=1 first, so its
                    # VectorE drain overlaps the m=0 matmuls
                    ms = (1, 0) if ps == NPASS - 1 and k == KCH - 1 else (0, 1)
                    for m in ms:
                        lhsT = wt_t[:, k * OUT + m * 128:k * OUT + (m + 1) * 128]
                        for n in range(nt):
                            nc.tensor.matmul(
                                psum[m * nt + n][:, :NW],
                                lhsT,
                                yk[:, n * NW:(n + 1) * NW],
                                start=(k == 0), stop=(k == KCH - 1))

                # prefetch the next pass, also on the single sync-lane
                # stream: queue order alone paces it behind everything
                # needed earlier, so it can never starve the current pass.
                if ps < NPASS - 1:
                    emit_dma(ps + 1, nc.sync, nc.sync)

                last = ps == NPASS - 1
                for m in range(2):
                    ot = o_pool.tile([128, 1024], BF16, tag="ostage",
                                     name=f"o{ps}_{m}")
                    for n in range(nt):
                        src = psum[m * nt + n][:, :NW]
                        dst = ot[:, n * NW:(n + 1) * NW]
                        if last and m == 1:
                            # tail: drain half the PSUM on the idle VectorE
                            nc.vector.tensor_copy(dst, src)
                        else:
                            nc.scalar.activation(
                                dst, src, mybir.ActivationFunctionType.Identity)
                    eng = nc.sync if (last and m == 1) else nc.scalar
                    eng.dma_start(out_d.ap()[m, :, off:off + fd], ot[:, :fd])
    nc.compile()
    _cached = nc
    return nc


def make_in_maps(x, wt):
    # [KCH, 128, OUT] -> [128, KCH*OUT], split into the three DMA tensors
    wtp = wt.transpose(1, 0, 2).reshape(128, KCH * OUT).astype(NPBF16)
    wt0 = np.ascontiguousarray(wtp[:, :W_A0 * OUT])
    wta = np.ascontiguousarray(wtp[:, W_A0 * OUT:W_A1 * OUT])
    wtb = np.ascontiguousarray(wtp[:, W_A1 * OUT:])
    bounds = np.concatenate([[0], np.cumsum(PASS_FD)])
    in_maps = []
    for b in range(B):
        xc = np.asarray(x[b], np.float32).reshape(C, PIX).astype(NPBF16)
        bufs = [np.concatenate([np.roll(xc, -t, axis=0),
                                np.roll(xc, -u, axis=0)])
                for t, u in zip(T_ROT, U_ROT)]
        # pass-major packing: per pass, buffer-major blocks; pass 0's group
        # A is laid out in compute order for the two-segment DMA
        ba = np.hstack([np.hstack([bufs[i][:, bounds[p]:bounds[p + 1]]
                                   for i in (B0A_ORDER if p == 0
                                             else range(NA))])
                        for p in range(NPASS)])
        bb = np.hstack([np.hstack([bf[:, bounds[p]:bounds[p + 1]]
                                   for bf in bufs[NA:]])
                        for p in range(NPASS)])
        in_maps.append({
            "wt0": wt0, "wta": wta, "wtb": wtb,
            "ba": np.ascontiguousarray(ba),
            "bb": np.ascontiguousarray(bb),
        })
    return in_maps


def assemble_out(res):
    outs = []
    for b in range(B):
        o = np.asarray(res.results[b]["out"]).astype(np.float32)
        outs.append(o.reshape(OUT, H, W))
    return np.stack(outs)


def kernel(x, fc_w):
    x = np.asarray(x, dtype=np.float32)
    fc_w = np.asarray(fc_w, dtype=np.float32)
    nc = _build_module()
    wt = build_wt(fc_w)
    res = run_bass_kernel_spmd(nc, make_in_maps(x, wt), list(range(NCORES)))
    return assemble_out(res)


# revision 26
# speedup vs baseline: 1.1525x; 1.1525x over previous
"""Trainium2 kernel for ChannelQuadLayer.

Per-pixel quadratic channel expansion + 1x1 conv:
    quad = x[:, ii] * x[:, jj]  (all 2080 upper-tri channel pairs)
    y    = concat([x, quad])    -> [B, 2144, H, W]
    out  = einsum('bchw,oc->bohw', y, fc_w)

Strategy (8 NeuronCores, batch-parallel, one sample per core):
  * The 2080 unordered channel pairs are exactly the cyclic diagonals
    d=0..32 of the 64-channel index ring: pairs {i, (i+d)%64}.
  * Host prepares 9 "rotation buffers" B_k = [roll(x,-t_k); roll(x,-u_k)]
    (128 partitions x 4096 pixels, bf16). One elementwise multiply of two
    such buffers yields TWO complete cyclic diagonals; a difference cover
    produces all diagonals 1..32 in 16 multiplies. Diagonal 0 (squares)
    and the linear rows come from buffer 0 (copy + self-multiply).
  * y-rows: 64 linear + 64 squares + 16*128 pair rows = 2176 = 17*128,
    an exact 17-chunk contraction. fc_w is permuted/padded to this row
    order on the host (duplicate pair rows get zero weight), cast bf16.
  * GEMM: out[256, 4096] = Wt[2176, 256]^T @ y[2176, 4096] on TensorE in
    bf16 (full PE rate, half the SBUF/HBM traffic of fp32), accumulating
    17 chunks into fp32 PSUM, k-outer so each y chunk is consumed right
    after its producer. The PE stream is the roofline (~58us at 2.4GHz).
  * Engine split keeps the PE gap-free: VectorE produces ALL y chunks
    (incl. chunk 0 via copy+self-mul), ScalarE only drains PSUM.
  * Startup is DMA-latency-bound and queue arbitration is per-packet
    (wide-row transfers crowd out narrow ones), so the ENTIRE input
    stream rides ONE sync-lane queue in exact compute-need order —
    queue order is priority, so a prefetch can never starve the current
    pass — with only the bulk chunk-6..16 weight block on the scalar
    lane. Dummy matmuls on a zeroed tile keep the PE busy (and its
    p-state ramping) through the initial DMA-latency window. Pass
    widths [512,1024,1024,1024,512]: small first pass starts the PE
    early, small last pass shortens the drain+writeback tail.
"""

import sys

sys.path.insert(0, "/opt/trn_rl_repo")

import numpy as np
import ml_dtypes

import concourse.bass as bass
import concourse.tile as tile
from concourse import bacc, mybir
from concourse.bass_utils import run_bass_kernel_spmd

B, C, H, W = 8, 64, 64, 64
PIX = H * W  # 4096
OUT = 256
NCORES = 8

# rotation difference cover: ops (i,j) give diagonals D(t_j-t_i) (top half)
# and D(u_j-u_i) (bottom half); together exactly {1..32}.
T_ROT = [0, 8, 22, 24, 42, 48, 49, 57, 60]
U_ROT = [0, 59, 16, 38, 55, 22, 30, 54, 35]
# group A = buffers 0-4, group B = buffers 5-8; A-only ops first so pass 0
# can start computing while group B is still in flight.
OPS_A = [(1, 3), (2, 3), (1, 4), (2, 4), (3, 4)]
OPS_B = [(4, 5), (1, 6), (2, 6), (6, 7), (0, 7), (4, 7),
         (5, 7), (2, 8), (3, 8), (5, 8), (6, 8)]
OPS = OPS_A + OPS_B
NB = len(T_ROT)        # 9 rotation buffers
NA, NBB = 5, 4         # buffers per group
KCH = 1 + len(OPS)     # 17 contraction chunks of 128 rows
PASS_FD = [512, 1024, 1024, 1024, 512]
assert sum(PASS_FD) == PIX
NPASS = len(PASS_FD)
NW = 512               # matmul free width (one PSUM bank)
# weight DMA split (chunk boundaries): chunk 0 alone so the PE can start
# after 65KB, chunks 1-5 / 6-16 as separate contiguous tensors
W_A0, W_A1 = 1, 1 + len(OPS_A)
# pass-0 group-A buffer order in DRAM = compute order, so the two
# segments [b0,b1,b3] and [b2,b4] are contiguous single transfers
B0A_ORDER = [0, 1, 3, 2, 4]
B0A_POS = {b: i for i, b in enumerate(B0A_ORDER)}

F32 = mybir.dt.float32
BF16 = mybir.dt.bfloat16
NPBF16 = ml_dtypes.bfloat16


def row_pairs():
    """Channel pair (c1, c2) for every global y row, or ('lin', c)."""
    rows = []
    for p in range(128):  # chunk 0
        rows.append(("lin", p) if p < 64 else (p - 64, p - 64))
    for (i, j) in OPS:
        for p in range(128):
            if p < 64:
                c1, c2 = (p + T_ROT[i]) % 64, (p + T_ROT[j]) % 64
            else:
                c1, c2 = (p - 64 + U_ROT[i]) % 64, (p - 64 + U_ROT[j]) % 64
            rows.append((min(c1, c2), max(c1, c2)))
    return rows


def build_wt(fc_w):
    """Permute fc_w [OUT, 2144] into Wt [KCH, 128, OUT] matching y rows."""
    ii, jj = np.triu_indices(C)
    pair2col = {(a, b): C + k for k, (a, b) in enumerate(zip(ii, jj))}
    wt = np.zeros((KCH * 128, OUT), np.float32)
    seen = set()
    for g, r in enumerate(row_pairs()):
        if r[0] == "lin":
            wt[g] = fc_w[:, r[1]]
        elif r not in seen:
            seen.add(r)
            wt[g] = fc_w[:, pair2col[r]]
    assert len(seen) == C * (C + 1) // 2
    return np.ascontiguousarray(wt.reshape(KCH, 128, OUT))


_cached = None


def _build_module():
    global _cached
    if _cached is not None:
        return _cached
    nc = bacc.Bacc("TRN2", target_bir_lowering=False, debug=False,
                   num_devices=NCORES)
    # grouped rotation buffers, pass-major so each pass is one contiguous DMA
    ba_d = nc.dram_tensor("ba", [128, NA * PIX], BF16, kind="ExternalInput")
    bb_d = nc.dram_tensor("bb", [128, NBB * PIX], BF16, kind="ExternalInput")
    # weights as three partition-major tensors so each transfer has wide
    # contiguous rows AND arrives in chunk-need order
    wt0_d = nc.dram_tensor("wt0", [128, W_A0 * OUT], BF16,
                           kind="ExternalInput")
    wta_d = nc.dram_tensor("wta", [128, (W_A1 - W_A0) * OUT], BF16,
                           kind="ExternalInput")
    wtb_d = nc.dram_tensor("wtb", [128, (KCH - W_A1) * OUT], BF16,
                           kind="ExternalInput")
    out_d = nc.dram_tensor("out", [2, 128, PIX], BF16, kind="ExternalOutput")

    with tile.TileContext(nc) as tc:
        with tc.tile_pool(name="wt", bufs=1) as wt_pool, \
             tc.tile_pool(name="warm", bufs=1) as warm_pool, \
             tc.tile_pool(name="ba", bufs=3) as ba_pool, \
             tc.tile_pool(name="bb", bufs=3) as bb_pool, \
             tc.tile_pool(name="y", bufs=8) as y_pool, \
             tc.tile_pool(name="ostage", bufs=4) as o_pool, \
             tc.tile_pool(name="psum", bufs=8, space="PSUM") as ps_pool:

            wt_t = wt_pool.tile([128, KCH * OUT], BF16, name="wtt")

            ba_t = [None] * NPASS
            bb_t = [None] * NPASS

            def emit_dma(ps, eng_a, eng_b):
                """Input DMAs for pass ps (group A / group B lanes)."""
                fd = PASS_FD[ps]
                offs = sum(PASS_FD[:ps])
                ba_t[ps] = ba_pool.tile([128, NA * 1024], BF16, tag="ba",
                                        name=f"ba{ps}")
                bb_t[ps] = bb_pool.tile([128, NBB * 1024], BF16, tag="bb",
                                        name=f"bb{ps}")
                a0, b0 = NA * offs, NBB * offs
                eng_a.dma_start(ba_t[ps][:, :NA * fd],
                                ba_d.ap()[:, a0:a0 + NA * fd])
                eng_b.dma_start(bb_t[ps][:, :NBB * fd],
                                bb_d.ap()[:, b0:b0 + NBB * fd])

            # DMA bandwidth is shared per-packet across queues and prefers
            # wide rows, so: ONE strictly-ordered input stream on the sync
            # lane (queue order = need order = perfect priority), with only
            # the two bulk weight blocks on the scalar lane, whose 2.5-5.6KB
            # rows hold their own in arbitration while pass-0 buffers flow.
            fd0 = PASS_FD[0]
            ba_t[0] = ba_pool.tile([128, NA * 1024], BF16, tag="ba", name="ba0")
            bb_t[0] = bb_pool.tile([128, NBB * 1024], BF16, tag="bb", name="bb0")

            nc.sync.dma_start(wt_t[:, :W_A0 * OUT], wt0_d.ap())
            nc.sync.dma_start(ba_t[0][:, :fd0], ba_d.ap()[:, :fd0])
            nc.sync.dma_start(ba_t[0][:, fd0:3 * fd0],
                              ba_d.ap()[:, fd0:3 * fd0])
            nc.sync.dma_start(wt_t[:, W_A0 * OUT:W_A1 * OUT], wta_d.ap())
            nc.sync.dma_start(ba_t[0][:, 3 * fd0:5 * fd0],
                              ba_d.ap()[:, 3 * fd0:5 * fd0])
            nc.sync.dma_start(bb_t[0][:, :NBB * fd0],
                              bb_d.ap()[:, :NBB * fd0])
            nc.scalar.dma_start(wt_t[:, W_A1 * OUT:], wtb_d.ap())

            # warm-up: dummy matmuls on a zeroed tile keep the PE busy
            # through the DMA-latency window before the first real chunk,
            # so the p-state ramp completes before real work arrives and
            # the first matmuls issue at full rate. Sized to end just as
            # the first real y chunk becomes ready (~11.5us).
            warm_t = warm_pool.tile([128, 512], BF16, name="warm")
            nc.gpsimd.memset(warm_t[:, :], 0)
            warm_ps = ps_pool.tile([128, NW], F32, tag="ps", name="warmps")
            for w in range(5):
                nc.tensor.matmul(warm_ps[:, :NW], warm_t[:, :128],
                                 warm_t[:, :NW], start=True, stop=True)
            for w in range(6):
                nc.tensor.matmul(warm_ps[:, :128], warm_t[:, :128],
                                 warm_t[:, :128], start=True, stop=True)

            for ps in range(NPASS):
                fd = PASS_FD[ps]
                off = sum(PASS_FD[:ps])
                nt = fd // NW
                bat, bbt = ba_t[ps], bb_t[ps]

                psum = [ps_pool.tile([128, NW], F32, tag="ps",
                                     name=f"ps{ps}_{g}")
                        for g in range(2 * nt)]

                for k in range(KCH):
                    yk = y_pool.tile([128, 1024], BF16, tag="y",
                                     name=f"y{ps}_{k}")
                    if k == 0:
                        # linear rows + squares, both from the resident b0
                        nc.vector.tensor_copy(yk[0:64, :fd], bat[0:64, :fd])
                        nc.vector.tensor_mul(yk[64:128, :fd],
                                             bat[64:128, :fd],
                                             bat[64:128, :fd])
                    else:
                        i, j = OPS[k - 1]

                        def src(i):
                            if i < NA:
                                p = B0A_POS[i] if ps == 0 else i
                                return bat[:, p * fd:(p + 1) * fd]
                            return bbt[:, (i - NA) * fd:(i - NA + 1) * fd]

                        nc.vector.tensor_mul(yk[:, :fd], src(i), src(j))
                    # final chunk of the final pass: m=1 first, so its
                    # VectorE drain overlaps the m=0 matmuls
                    ms = (1, 0) if ps == NPASS - 1 and k == KCH - 1 else (0, 1)
                    for m in ms:
                        lhsT = wt_t[:, k * OUT + m * 128:k * OUT + (m + 1) * 128]
                        for n in range(nt):
                            nc.tensor.matmul(
                                psum[m * nt + n][:, :NW],
                                lhsT,
                                yk[:, n * NW:(n + 1) * NW],
                                start=(k == 0), stop=(k == KCH - 1))

                # prefetch the next pass, also on the single sync-lane
                # stream: queue order alone paces it behind everything
                # needed earlier, so it can never starve the current pass.
                if ps < NPASS - 1:
                    emit_dma(ps + 1, nc.sync, nc.sync)

                last = ps == NPASS - 1
                for m in range(2):
                    ot = o_pool.tile([128, 1024], BF16, tag="ostage",
                                     name=f"o{ps}_{m}")
                    for n in range(nt):
                        src = psum[m * nt + n][:, :NW]
                        dst = ot[:, n * NW:(n + 1) * NW]
                        if last and m == 1:
                            # tail: drain half the PSUM on the idle VectorE
                            nc.vector.tensor_copy(dst, src)
                        else:
                            nc.scalar.activation(
                                dst, src, mybir.ActivationFunctionType.Identity)
                    eng = nc.sync if (last and m == 1) else nc.scalar
                    eng.dma_start(out_d.ap()[m, :, off:off + fd], ot[:, :fd])
    nc.compile()
    _cached = nc
    return nc


def make_in_maps(x, wt):
    # [KCH, 128, OUT] -> [128, KCH*OUT], split into the three DMA tensors
    wtp = wt.transpose(1, 0, 2).reshape(128, KCH * OUT).astype(NPBF16)
    wt0 = np.ascontiguousarray(wtp[:, :W_A0 * OUT])
    wta = np.ascontiguousarray(wtp[:, W_A0 * OUT:W_A1 * OUT])
    wtb = np.ascontiguousarray(wtp[:, W_A1 * OUT:])
    bounds = np.concatenate([[0], np.cumsum(PASS_FD)])
    in_maps = []
    for b in range(B):
        xc = np.asarray(x[b], np.float32).reshape(C, PIX).astype(NPBF16)
        bufs = [np.concatenate([np.roll(xc, -t, axis=0),
                                np.roll(xc, -u, axis=0)])
                for t, u in zip(T_ROT, U_ROT)]
        # pass-major packing: per pass, buffer-major blocks; pass 0's group
        # A is laid out in compute order for the two-segment DMA
        ba = np.hstack([np.hstack([bufs[i][:, bounds[p]:bounds[p + 1]]
                                   for i in (B0A_ORDER if p == 0
                                             else range(NA))])
                        for p in range(NPASS)])
        bb = np.hstack([np.hstack([bf[:, bounds[p]:bounds[p + 1]]
                                   for bf in bufs[NA:]])
                        for p in range(NPASS)])
        in_maps.append({
            "wt0": wt0, "wta": wta, "wtb": wtb,
            "ba": np.ascontiguousarray(ba),
            "bb": np.ascontiguousarray(bb),
        })
    return in_maps


def assemble_out(res):
    outs = []
    for b in range(B):
        o = np.asarray(res.results[b]["out"]).astype(np.float32)
        outs.append(o.reshape(OUT, H, W))
    return np.stack(outs)


def kernel(x, fc_w):
    x = np.asarray(x, dtype=np.float32)
    fc_w = np.asarray(fc_w, dtype=np.float32)
    nc = _build_module()
    wt = build_wt(fc_w)
    res = run_bass_kernel_spmd(nc, make_in_maps(x, wt), list(range(NCORES)))
    return assemble_out(res)


# revision 31
# speedup vs baseline: 1.1888x; 1.0315x over previous
"""Trainium2 kernel for ChannelQuadLayer.

Per-pixel quadratic channel expansion + 1x1 conv:
    quad = x[:, ii] * x[:, jj]  (all 2080 upper-tri channel pairs)
    y    = concat([x, quad])    -> [B, 2144, H, W]
    out  = einsum('bchw,oc->bohw', y, fc_w)

Strategy (8 NeuronCores, batch-parallel, one sample per core):
  * The 2080 unordered channel pairs are exactly the cyclic diagonals
    d=0..32 of the 64-channel index ring: pairs {i, (i+d)%64}.
  * Host prepares 9 "rotation buffers" B_k = [roll(x,-t_k); roll(x,-u_k)]
    (128 partitions x 4096 pixels, bf16). One elementwise multiply of two
    such buffers yields TWO complete cyclic diagonals; a difference cover
    produces all diagonals 1..32 in 16 multiplies. Diagonal 0 (squares)
    and the linear rows come from buffer 0 (copy + self-multiply).
  * y-rows: 64 linear + 64 squares + 16*128 pair rows = 2176 = 17*128,
    an exact 17-chunk contraction. fc_w is permuted/padded to this row
    order on the host (duplicate pair rows get zero weight), cast bf16.
  * GEMM: out[256, 4096] = Wt[2176, 256]^T @ y[2176, 4096] on TensorE in
    bf16 (full PE rate, half the SBUF/HBM traffic of fp32), accumulating
    17 chunks into fp32 PSUM, k-outer so each y chunk is consumed right
    after its producer. The PE stream is the roofline (~58us at 2.4GHz).
  * Engine split keeps the PE gap-free: VectorE produces ALL y chunks
    (incl. chunk 0 via copy+self-mul), ScalarE only drains PSUM.
  * Startup is DMA-latency-bound and queue arbitration is per-packet
    (wide-row transfers crowd out narrow ones), so the ENTIRE input
    stream rides ONE sync-lane queue in exact compute-need order —
    queue order is priority, so a prefetch can never starve the current
    pass — with only the bulk chunk-6..16 weight block on the scalar
    lane. Dummy matmuls on a zeroed tile keep the PE busy (and its
    p-state ramping) through the initial DMA-latency window. Pass
    widths [512,1024,1024,1024,512]: small first pass starts the PE
    early, small last pass shortens the drain+writeback tail.
"""

import sys

sys.path.insert(0, "/opt/trn_rl_repo")

import numpy as np
import ml_dtypes

import concourse.bass as bass
import concourse.tile as tile
from concourse import bacc, mybir
from concourse.bass_utils import run_bass_kernel_spmd

B, C, H, W = 8, 64, 64, 64
PIX = H * W  # 4096
OUT = 256
NCORES = 8

# rotation difference cover: ops (i,j) give diagonals D(t_j-t_i) (top half)
# and D(u_j-u_i) (bottom half); together exactly {1..32}.
T_ROT = [0, 8, 22, 24, 42, 48, 49, 57, 60]
U_ROT = [0, 59, 16, 38, 55, 22, 30, 54, 35]
# group A = buffers 0-4, group B = buffers 5-8; A-only ops first so pass 0
# can start computing while group B is still in flight.
OPS_A = [(1, 3), (2, 3), (1, 4), (2, 4), (3, 4)]
OPS_B = [(4, 5), (1, 6), (2, 6), (6, 7), (0, 7), (4, 7),
         (5, 7), (2, 8), (3, 8), (5, 8), (6, 8)]
OPS = OPS_A + OPS_B
NB = len(T_ROT)        # 9 rotation buffers
NA, NBB = 5, 4         # buffers per group
KCH = 1 + len(OPS)     # 17 contraction chunks of 128 rows
PASS_FD = [512, 1024, 1024, 1024, 512]
assert sum(PASS_FD) == PIX
NPASS = len(PASS_FD)
NW = 512               # matmul free width (one PSUM bank)
# weight DMA split (chunk boundaries): chunk 0 alone so the PE can start
# after 65KB, chunks 1-5 / 6-16 as separate contiguous tensors
W_A0, W_A1 = 1, 1 + len(OPS_A)
# pass-0 group-A buffer order in DRAM = compute order, so the two
# segments [b0,b1,b3] and [b2,b4] are contiguous single transfers
B0A_ORDER = [0, 1, 3, 2, 4]
B0A_POS = {b: i for i, b in enumerate(B0A_ORDER)}

F32 = mybir.dt.float32
BF16 = mybir.dt.bfloat16
NPBF16 = ml_dtypes.bfloat16


def row_pairs():
    """Channel pair (c1, c2) for every global y row, or ('lin', c)."""
    rows = []
    for p in range(128):  # chunk 0
        rows.append(("lin", p) if p < 64 else (p - 64, p - 64))
    for (i, j) in OPS:
        for p in range(128):
            if p < 64:
                c1, c2 = (p + T_ROT[i]) % 64, (p + T_ROT[j]) % 64
            else:
                c1, c2 = (p - 64 + U_ROT[i]) % 64, (p - 64 + U_ROT[j]) % 64
            rows.append((min(c1, c2), max(c1, c2)))
    return rows


def build_wt(fc_w):
    """Permute fc_w [OUT, 2144] into Wt [KCH, 128, OUT] matching y rows."""
    ii, jj = np.triu_indices(C)
    pair2col = {(a, b): C + k for k, (a, b) in enumerate(zip(ii, jj))}
    wt = np.zeros((KCH * 128, OUT), np.float32)
    seen = set()
    for g, r in enumerate(row_pairs()):
        if r[0] == "lin":
            wt[g] = fc_w[:, r[1]]
        elif r not in seen:
            seen.add(r)
            wt[g] = fc_w[:, pair2col[r]]
    assert len(seen) == C * (C + 1) // 2
    return np.ascontiguousarray(wt.reshape(KCH, 128, OUT))


_cached = None


def _build_module():
    global _cached
    if _cached is not None:
        return _cached
    nc = bacc.Bacc("TRN2", target_bir_lowering=False, debug=False,
                   num_devices=NCORES)
    # grouped rotation buffers, pass-major so each pass is one contiguous DMA
    ba_d = nc.dram_tensor("ba", [128, NA * PIX], BF16, kind="ExternalInput")
    bb_d = nc.dram_tensor("bb", [128, NBB * PIX], BF16, kind="ExternalInput")
    # "head": weights for chunks 0-5 + the five pass-0 group-A buffers as
    # ONE wide-row tensor — early DMA is packet-latency-bound (~1.3us per
    # 128-row transfer regardless of size), so one 8KB-row transfer
    # delivers everything the first six chunks need at once
    HEADW = W_A1 * OUT + NA * PASS_FD[0]
    head_d = nc.dram_tensor("head", [128, HEADW], BF16, kind="ExternalInput")
    # full weights again for passes 1+ (chunks 0-5 duplicated; DMA slack
    # mid-kernel is free), split so each piece has wide contiguous rows
    wta_d = nc.dram_tensor("wta", [128, W_A1 * OUT], BF16,
                           kind="ExternalInput")
    wtb_d = nc.dram_tensor("wtb", [128, (KCH - W_A1) * OUT], BF16,
                           kind="ExternalInput")
    out_d = nc.dram_tensor("out", [2, 128, PIX], BF16, kind="ExternalOutput")

    with tile.TileContext(nc) as tc:
        with tc.tile_pool(name="wt", bufs=1) as wt_pool, \
             tc.tile_pool(name="head", bufs=1) as head_pool, \
             tc.tile_pool(name="warm", bufs=1) as warm_pool, \
             tc.tile_pool(name="ba", bufs=3) as ba_pool, \
             tc.tile_pool(name="bb", bufs=3) as bb_pool, \
             tc.tile_pool(name="y", bufs=8) as y_pool, \
             tc.tile_pool(name="ostage", bufs=4) as o_pool, \
             tc.tile_pool(name="psum", bufs=8, space="PSUM") as ps_pool:

            wt_t = wt_pool.tile([128, KCH * OUT], BF16, name="wtt")

            ba_t = [None] * NPASS
            bb_t = [None] * NPASS

            def emit_dma(ps, eng_a, eng_b):
                """Input DMAs for pass ps (group A / group B lanes)."""
                fd = PASS_FD[ps]
                offs = sum(PASS_FD[:ps])
                ba_t[ps] = ba_pool.tile([128, NA * 1024], BF16, tag="ba",
                                        name=f"ba{ps}")
                bb_t[ps] = bb_pool.tile([128, NBB * 1024], BF16, tag="bb",
                                        name=f"bb{ps}")
                a0, b0 = NA * offs, NBB * offs
                eng_a.dma_start(ba_t[ps][:, :NA * fd],
                                ba_d.ap()[:, a0:a0 + NA * fd])
                eng_b.dma_start(bb_t[ps][:, :NBB * fd],
                                bb_d.ap()[:, b0:b0 + NBB * fd])

            # DMA bandwidth is shared per-packet across queues and prefers
            # wide rows, so: ONE strictly-ordered input stream on the sync
            # lane (queue order = need order = perfect priority), with only
            # the two bulk weight blocks on the scalar lane, whose 2.5-5.6KB
            # rows hold their own in arbitration while pass-0 buffers flow.
            fd0 = PASS_FD[0]
            head_t = head_pool.tile([128, HEADW], BF16, name="head")
            bb_t[0] = bb_pool.tile([128, NBB * 1024], BF16, tag="bb", name="bb0")

            nc.sync.dma_start(head_t[:, :], head_d.ap())
            nc.sync.dma_start(bb_t[0][:, :NBB * fd0],
                              bb_d.ap()[:, :NBB * fd0])
            nc.sync.dma_start(wt_t[:, :W_A1 * OUT], wta_d.ap())
            nc.scalar.dma_start(wt_t[:, W_A1 * OUT:], wtb_d.ap())

            # warm-up: dummy matmuls on a zeroed tile keep the PE busy
            # through the DMA-latency window before the first real chunk,
            # so the p-state ramp completes before real work arrives and
            # the first matmuls issue at full rate. Sized to end just as
            # the first real y chunk becomes ready (~11.5us).
            warm_t = warm_pool.tile([128, 512], BF16, name="warm")
            nc.gpsimd.memset(warm_t[:, :], 0)
            warm_ps = ps_pool.tile([128, NW], F32, tag="ps", name="warmps")
            for w in range(8):
                nc.tensor.matmul(warm_ps[:, :NW], warm_t[:, :128],
                                 warm_t[:, :NW], start=True, stop=True)
            for w in range(4):
                nc.tensor.matmul(warm_ps[:, :128], warm_t[:, :128],
                                 warm_t[:, :128], start=True, stop=True)

            for ps in range(NPASS):
                fd = PASS_FD[ps]
                off = sum(PASS_FD[:ps])
                nt = fd // NW
                bat, bbt = ba_t[ps], bb_t[ps]

                def src(i):
                    """SBUF slice holding rotation buffer i for this pass."""
                    if i < NA:
                        if ps == 0:
                            p = W_A1 * OUT + B0A_POS[i] * fd
                            return head_t[:, p:p + fd]
                        return bat[:, i * fd:(i + 1) * fd]
                    return bbt[:, (i - NA) * fd:(i - NA + 1) * fd]

                psum = [ps_pool.tile([128, NW], F32, tag="ps",
                                     name=f"ps{ps}_{g}")
                        for g in range(2 * nt)]

                for k in range(KCH):
                    yk = y_pool.tile([128, 1024], BF16, tag="y",
                                     name=f"y{ps}_{k}")
                    if k == 0:
                        # linear rows + squares, both from the resident b0
                        b0s = src(0)
                        nc.vector.tensor_copy(yk[0:64, :fd], b0s[0:64, :])
                        nc.vector.tensor_mul(yk[64:128, :fd],
                                             b0s[64:128, :],
                                             b0s[64:128, :])
                    else:
                        i, j = OPS[k - 1]
                        nc.vector.tensor_mul(yk[:, :fd], src(i), src(j))
                    # final chunk of the final pass: m=1 first, so its
                    # VectorE drain overlaps the m=0 matmuls
                    ms = (1, 0) if ps == NPASS - 1 and k == KCH - 1 else (0, 1)
                    for m in ms:
                        wsrc = head_t if ps == 0 and k < W_A1 else wt_t
                        lhsT = wsrc[:, k * OUT + m * 128:k * OUT + (m + 1) * 128]
                        for n in range(nt):
                            nc.tensor.matmul(
                                psum[m * nt + n][:, :NW],
                                lhsT,
                                yk[:, n * NW:(n + 1) * NW],
                                start=(k == 0), stop=(k == KCH - 1))

                # prefetch the next pass, also on the single sync-lane
                # stream: queue order alone paces it behind everything
                # needed earlier, so it can never starve the current pass.
                if ps < NPASS - 1:
                    emit_dma(ps + 1, nc.sync, nc.sync)

                last = ps == NPASS - 1
                for m in range(2):
                    ot = o_pool.tile([128, 1024], BF16, tag="ostage",
                                     name=f"o{ps}_{m}")
                    for n in range(nt):
                        src = psum[m * nt + n][:, :NW]
                        dst = ot[:, n * NW:(n + 1) * NW]
                        if last and m == 1:
                            # tail: drain half the PSUM on the idle VectorE
                            nc.vector.tensor_copy(dst, src)
                        else:
                            nc.scalar.activation(
                                dst, src, mybir.ActivationFunctionType.Identity)
                    eng = nc.sync if (last and m == 1) else nc.scalar
                    eng.dma_start(out_d.ap()[m, :, off:off + fd], ot[:, :fd])
    nc.compile()
    _cached = nc
    return nc


def make_in_maps(x, wt):
    # [KCH, 128, OUT] -> [128, KCH*OUT], split into the DMA tensors
    wtp = wt.transpose(1, 0, 2).reshape(128, KCH * OUT).astype(NPBF16)
    wta = np.ascontiguousarray(wtp[:, :W_A1 * OUT])
    wtb = np.ascontiguousarray(wtp[:, W_A1 * OUT:])
    bounds = np.concatenate([[0], np.cumsum(PASS_FD)])
    in_maps = []
    for b in range(B):
        xc = np.asarray(x[b], np.float32).reshape(C, PIX).astype(NPBF16)
        bufs = [np.concatenate([np.roll(xc, -t, axis=0),
                                np.roll(xc, -u, axis=0)])
                for t, u in zip(T_ROT, U_ROT)]
        # head: chunk 0-5 weights + pass-0 group A in compute order
        head = np.hstack([wta] + [bufs[i][:, :bounds[1]] for i in B0A_ORDER])
        # pass-major packing for passes 1+: per pass, buffer-major blocks
        ba = np.hstack([np.hstack([bufs[i][:, bounds[p]:bounds[p + 1]]
                                   for i in range(NA)])
                        for p in range(NPASS)])
        bb = np.hstack([np.hstack([bf[:, bounds[p]:bounds[p + 1]]
                                   for bf in bufs[NA:]])
                        for p in range(NPASS)])
        in_maps.append({
            "head": np.ascontiguousarray(head),
            "wta": wta, "wtb": wtb,
            "ba": np.ascontiguousarray(ba),
            "bb": np.ascontiguousarray(bb),
        })
    return in_maps


def assemble_out(res):
    outs = []
    for b in range(B):
        o = np.asarray(res.results[b]["out"]).astype(np.float32)
        outs.append(o.reshape(OUT, H, W))
    return np.stack(outs)


def kernel(x, fc_w):
    x = np.asarray(x, dtype=np.float32)
    fc_w = np.asarray(fc_w, dtype=np.float32)
    nc = _build_module()
    wt = build_wt(fc_w)
    res = run_bass_kernel_spmd(nc, make_in_maps(x, wt), list(range(NCORES)))
    return assemble_out(res)
